# revision 2
# baseline (speedup 1.0000x reference)
"""Causal multi-head self-attention (B=4, S=2048, D=1024, H=16) on 8 TRN2
NeuronCores.

Sharding: core c = (batch b = c//2, head-half = c%2). Each core computes, for
its batch and its 8 heads: QKV projections (+RoPE via host-permuted weights
and a signed sin table), causal softmax attention, and a row-sharded output
projection. The host sums the two partial y's per batch.

Device layouts (per core):
  qT,kT: [128, 4, 2048]  chunk hc = heads (2hc, 2hc+1); within a head's 64
         rows: [even dims (32) | odd dims (32)] — RoPE pairs at partition
         offset +32, applied straight out of the projection PSUM.
  v:     [128, 16, 8, 65] = [t % 128, t//128, head, dim+ones]; the 65th
         column of ones makes the attention matmul emit the softmax
         denominator as PSUM row 64.
  scores are built transposed (S.T[t, s]) so exp(S.T) feeds the AV matmul as
  the moving operand with no transposes anywhere: out.T = v.T @ P.T.
  All matmuls run as float32r (fp32 data, reduced-precision multiply at
  1 cycle/row for moving dims >= 256).

Structure notes:
  - PSUM pools are global (tags p1 / sc / pa = 2+4+2 banks): no phase
    boundary PSUM stalls. SBUF pools phase (weights/x close before outT/Wo
    pools open) to fit the 192KB/partition budget.
  - Head-pair-outer attention; each pair's denominators batch into one 8-row
    reciprocal whose DRAM broadcast round-trip trails one head-pair behind.
  - Diagonal score/AV matmuls are column-narrowed to the causal range, which
    also removes any need to zero the masked region of exp tiles.
"""

import numpy as np

B, S, D = 4, 2048, 1024
NUM_HEADS = 16
THETA = 10000.0
DH = 64
N_CORES = 8
P = 128

_CACHE = {}


def build_nc():
    """Build the single-core SPMD Bass program (identical on all 8 cores)."""
    import concourse.mybir as mybir
    import concourse.tile as tile
    from concourse import bacc
    from concourse.bass import ts

    F32 = mybir.dt.float32
    F32R = mybir.dt.float32r
    Act = mybir.ActivationFunctionType

    def r(ap):
        return ap.bitcast(F32R)

    nc = bacc.Bacc(trn_type="TRN2")
    xT_d = nc.dram_tensor("xT", [D, S], F32R, kind="ExternalInput")
    wqT_d = nc.dram_tensor("wqT", [D, 512], F32R, kind="ExternalInput")
    wkT_d = nc.dram_tensor("wkT", [D, 512], F32R, kind="ExternalInput")
    wvT_d = nc.dram_tensor("wvT", [D, 512], F32R, kind="ExternalInput")
    woT_d = nc.dram_tensor("woT", [512, D], F32R, kind="ExternalInput")
    cosT_d = nc.dram_tensor("cosT", [P, S], F32, kind="ExternalInput")
    sinT_d = nc.dram_tensor("sinT", [P, S], F32, kind="ExternalInput")
    tri_d = nc.dram_tensor("tri", [P, P], F32, kind="ExternalInput")
    y_d = nc.dram_tensor("y", [S, D], F32, kind="ExternalOutput")

    xT3 = xT_d.ap().rearrange("(kc p) s -> p kc s", p=P)     # [128, 8, 2048]
    wq3 = wqT_d.ap().rearrange("(kc p) j -> p kc j", p=P)    # [128, 8, 512]
    wk3 = wkT_d.ap().rearrange("(kc p) j -> p kc j", p=P)
    wv3 = wvT_d.ap().rearrange("(kc p) j -> p kc j", p=P)
    wo3 = woT_d.ap().rearrange("(jc p) i -> p jc i", p=P)    # [128, 4, 1024]
    y_ap = y_d.ap()

    with tile.TileContext(nc) as tc:
        with tc.tile_pool(name="pers", bufs=1) as pers:
            qT = pers.tile([P, 4, S], F32R)
            kT = pers.tile([P, 4, S], F32R)
            vA = pers.tile([P, 16, 8, 65], F32R)

            # ---- Phase 1a: q/k projections + RoPE ----
            with (
                tc.tile_pool(name="tab", bufs=1) as tab,
                tc.tile_pool(name="w1", bufs=1) as w1,
                tc.tile_pool(name="x1", bufs=2) as x1,
                tc.tile_pool(name="tmp1", bufs=2) as tmp1,
                tc.tile_pool(name="ps1", bufs=4, space="PSUM") as psA,
            ):
                wq_s = w1.tile([P, 8, 512], F32R)
                wk_s = w1.tile([P, 8, 512], F32R)
                cosb = tab.tile([P, S], F32)
                sinb = tab.tile([P, S], F32)
                for kc in range(8):  # chunked so kc=0 arrives early
                    nc.sync.dma_start(wq_s[:, kc, :], wq3[:, kc, :])
                    nc.sync.dma_start(wk_s[:, kc, :], wk3[:, kc, :])
                nc.sync.dma_start(cosb[:], cosT_d.ap())
                nc.sync.dma_start(sinb[:], sinT_d.ap())

                def rope(pq, dst2d, sls):
                    # tA = proj * cos (full width); tBs = 32-row-swapped proj
                    # times the SIGNED sin table (+sin top rows, -sin bottom
                    # rows); combine with one full-width add: r = tA + tBs
                    tA = tmp1.tile([P, 512], F32, tag="tA")
                    nc.vector.tensor_mul(tA[:], pq[:], cosb[:, sls])
                    tBs = tmp1.tile([P, 512], F32, tag="tBs")
                    for hb in (0, 64):
                        nc.vector.tensor_mul(
                            tBs[hb : hb + 32, :],
                            pq[hb + 32 : hb + 64, :], sinb[hb + 32 : hb + 64, sls],
                        )
                        nc.vector.tensor_mul(
                            tBs[hb + 32 : hb + 64, :],
                            pq[hb : hb + 32, :], sinb[hb : hb + 32, sls],
                        )
                    nc.vector.tensor_add(dst2d, tA[:], tBs[:])

                for sl in range(4):
                    sls = ts(sl, 512)
                    xs = x1.tile([P, 8, 512], F32R, tag="xs")
                    for kc in range(8):
                        nc.sync.dma_start(xs[:, kc, :], xT3[:, kc, sls])
                    for jc in range(4):
                        pq = psA.tile([P, 512], F32, tag="p1")
                        for kc in range(8):
                            nc.tensor.matmul(
                                pq[:], r(wq_s[:, kc, ts(jc, P)]), r(xs[:, kc, :]),
                                start=(kc == 0), stop=(kc == 7),
                            )
                        rope(pq, qT[:, jc, sls], sls)
                        pk = psA.tile([P, 512], F32, tag="p1")
                        for kc in range(8):
                            nc.tensor.matmul(
                                pk[:], r(wk_s[:, kc, ts(jc, P)]), r(xs[:, kc, :]),
                                start=(kc == 0), stop=(kc == 7),
                            )
                        rope(pk, kT[:, jc, sls], sls)

            # ---- Phase 1b: v projection ----
            with (
                tc.tile_pool(name="w2", bufs=1) as w2,
                tc.tile_pool(name="x2", bufs=2) as x2,
                tc.tile_pool(name="ps2", bufs=4, space="PSUM") as psA,
            ):
                wv_s = w2.tile([P, 8, 512], F32R)
                nc.sync.dma_start(wv_s[:], wv3)
                # f32r memset isn't a legal ISA value type; write the ones
                # columns via tensor_copy from a small fp32 tile instead
                ones8 = w2.tile([P, 8], F32)
                nc.vector.memset(ones8[:], 1.0)
                for t16 in range(16):
                    nc.vector.tensor_copy(vA[:, t16, :, 64:65], ones8.unsqueeze(2))
                for sl in range(4):
                    xs2 = x2.tile([P, 8, 512], F32R, tag="xs2")
                    nc.sync.dma_start(xs2[:], xT3[:, :, ts(sl, 512)])
                    for t4i in range(4):
                        pv = psA.tile([P, 512], F32, tag="p1")
                        for kc in range(8):
                            nc.tensor.matmul(
                                pv[:], r(xs2[:, kc, ts(t4i, P)]), r(wv_s[:, kc, :]),
                                start=(kc == 0), stop=(kc == 7),
                            )
                        nc.vector.tensor_copy(
                            vA[:, sl * 4 + t4i, :, 0:64],
                            pv.rearrange("p (h c) -> p h c", h=8),
                        )

            # ---- Phase 2: attention, head-pair outer ----
            with (
                tc.tile_pool(name="wo", bufs=1) as wo,
                tc.tile_pool(name="outp", bufs=1) as outp,
                tc.tile_pool(name="trip", bufs=1) as trip,
                tc.tile_pool(name="ptp", bufs=4) as ptp,
                tc.tile_pool(name="rcp", bufs=3) as rcp,
                tc.tile_pool(name="rbp", bufs=4) as rbp,
                tc.tile_pool(name="ysb", bufs=2) as ysb,
                tc.tile_pool(name="drm", bufs=2, space="DRAM") as drm,
            ):
                _psB_cm = tc.tile_pool(name="psB", bufs=2, space="PSUM")
                _psC_cm = tc.tile_pool(name="psC", bufs=2, space="PSUM")
                psB = _psB_cm.__enter__()
                psC = _psC_cm.__enter__()
                wo_s = wo.tile([P, 4, D], F32R)
                nc.sync.dma_start(wo_s[:], wo3)
                outT = outp.tile([P, 4, S], F32R)
                trib = trip.tile([P, P], F32)
                nc.sync.dma_start(trib[:], tri_d.ap())

                den_tiles = {}

                def attention_pair(hc):
                    den_d = drm.tile([8, 512], F32, tag="dend")
                    den_tiles[hc] = den_d
                    for j in range(4):
                        # both heads' scores/exp/AV share paired [*, 1024]
                        # tiles: head0 in cols 0:512, head1 in 512:1024
                        pa = psC.tile([65, 1024], F32, tag="pa")
                        last = 4 * j + 3
                        for i in range(last + 1):
                            m = i - 4 * j
                            w0 = max(m, 0) * P   # first causal col in the 512
                            sc = psB.tile([P, 1024], F32, tag="sc")
                            nc.tensor.matmul(
                                sc[:, w0:512], r(kT[0:64, hc, ts(i, P)]),
                                r(qT[0:64, hc, j * 512 + w0 : (j + 1) * 512]),
                                start=True, stop=True,
                            )
                            nc.tensor.matmul(
                                sc[:, 512 + w0 : 1024], r(kT[64:P, hc, ts(i, P)]),
                                r(qT[64:P, hc, j * 512 + w0 : (j + 1) * 512]),
                                start=True, stop=True,
                            )
                            pt = ptp.tile([P, 1024], F32R, tag="pt")
                            if m < 0:
                                nc.scalar.activation(pt[:], sc[:], Act.Exp)
                            else:
                                nc.scalar.activation(
                                    pt[:, w0:512], sc[:, w0:512], Act.Exp
                                )
                                nc.scalar.activation(
                                    pt[:, 512 + w0 : 1024],
                                    sc[:, 512 + w0 : 1024], Act.Exp,
                                )
                                nc.vector.tensor_mul(
                                    pt[:, w0 : w0 + P], pt[:, w0 : w0 + P], trib[:]
                                )
                                nc.vector.tensor_mul(
                                    pt[:, 512 + w0 : 512 + w0 + P],
                                    pt[:, 512 + w0 : 512 + w0 + P], trib[:],
                                )
                            nc.tensor.matmul(
                                pa[:, w0:512], r(vA[:, i, 2 * hc, :]),
                                r(pt[:, w0:512]),
                                start=(i == 0), stop=(i == last),
                            )
                            nc.tensor.matmul(
                                pa[:, 512 + w0 : 1024], r(vA[:, i, 2 * hc + 1, :]),
                                r(pt[:, 512 + w0 : 1024]),
                                start=(i == 0), stop=(i == last),
                            )
                        # release pa quickly: unnormalized out rows and
                        # denominator rows (both DVE; ScalarE paces the exp)
                        for h01 in range(2):
                            hb = h01 * 64
                            cs0 = h01 * 512
                            nc.vector.tensor_copy(
                                outT[hb : hb + 64, hc, ts(j, 512)],
                                pa[0:64, cs0 : cs0 + 512],
                            )
                            srow = rcp.tile([1, 512], F32, tag="srow")
                            nc.vector.tensor_copy(srow[:], pa[64:65, cs0 : cs0 + 512])
                            nc.sync.dma_start(
                                den_d[j * 2 + h01 : j * 2 + h01 + 1, :], srow[:]
                            )

                def epilogue_pair(hc, p3=None):
                    # batched denominators: one 8-row reciprocal, broadcast
                    # rows back through DRAM, divide in place. When p3 is
                    # set (last pair), interleave each j-block's divisions
                    # with that block's output-projection tiles.
                    den_d = den_tiles[hc]
                    den_sb = rcp.tile([8, 512], F32, tag="densb")
                    nc.sync.dma_start(den_sb[:], den_d[:])
                    rec8 = rcp.tile([8, 512], F32, tag="rec8")
                    nc.vector.reciprocal(rec8[:], den_sb[:])
                    rec_d = drm.tile([8, 512], F32, tag="recd")
                    nc.sync.dma_start(rec_d[:], rec8[:])
                    for j in range(4):
                        for h01 in range(2):
                            rb = rbp.tile([P, 512], F32, tag="rb")
                            row = j * 2 + h01
                            hb = h01 * 64
                            nc.sync.dma_start(
                                rb[hb : hb + 64, :],
                                rec_d[row : row + 1, :].broadcast_to((64, 512)),
                            )
                            nc.vector.tensor_mul(
                                outT[hb : hb + 64, hc, ts(j, 512)],
                                outT[hb : hb + 64, hc, ts(j, 512)],
                                rb[hb : hb + 64, :],
                            )
                        if p3 is not None:
                            p3(j)

                # epilogues trail one head-pair behind so their DMA round-trip
                # latency hides under the next pair's dense compute; the last
                # pair's divisions interleave with the output projection
                attention_pair(0)
                for hc in range(1, 4):
                    attention_pair(hc)
                    epilogue_pair(hc - 1)
                _psC_cm.__exit__(None, None, None)
                _psB_cm.__exit__(None, None, None)

                # ---- Phase 3: output projection y = outT.T @ woT ----
                ps3 = tc.tile_pool(name="ps3", bufs=2, space="PSUM")
                ps3p = ps3.__enter__()

                def p3_group(j):
                    for st in range(4 * j, 4 * j + 4):
                        py0 = ps3p.tile([P, 512], F32, tag="py0")
                        py1 = ps3p.tile([P, 512], F32, tag="py1")
                        for jc in range(4):
                            nc.tensor.matmul(
                                py0[:], r(outT[:, jc, ts(st, P)]),
                                r(wo_s[:, jc, 0:512]),
                                start=(jc == 0), stop=(jc == 3),
                            )
                        for jc in range(4):
                            nc.tensor.matmul(
                                py1[:], r(outT[:, jc, ts(st, P)]),
                                r(wo_s[:, jc, 512:D]),
                                start=(jc == 0), stop=(jc == 3),
                            )
                        yo0 = ysb.tile([P, 512], F32, tag="yo0")
                        yo1 = ysb.tile([P, 512], F32, tag="yo1")
                        nc.scalar.copy(yo0[:], py0[:])
                        nc.scalar.copy(yo1[:], py1[:])
                        nc.sync.dma_start(y_ap[ts(st, P), 0:512], yo0[:])
                        nc.sync.dma_start(y_ap[ts(st, P), 512:D], yo1[:])

                epilogue_pair(3)
                for _j in range(4):
                    p3_group(_j)
                ps3.__exit__(None, None, None)

    nc.compile()
    return nc


def prep_core_inputs(x, token_ids, Wq, Wk, Wv, Wo, core):
    b, half = divmod(core, 2)
    rows = []
    for h in range(half * 8, half * 8 + 8):
        base = h * DH
        rows.extend(base + np.arange(0, DH, 2))
        rows.extend(base + np.arange(1, DH, 2))
    rows = np.asarray(rows)
    cols = np.arange(half * 512, half * 512 + 512)

    f32 = np.float32
    inv = THETA ** (-np.arange(0, DH, 2, dtype=np.float64) / DH)
    ang = np.asarray(token_ids, dtype=np.float64)[None, :] * inv[:, None]
    cosT = np.tile(np.cos(ang), (4, 1)).astype(f32)
    sin_block = np.concatenate([np.sin(ang), -np.sin(ang)], axis=0)
    sinT = np.tile(sin_block, (2, 1)).astype(f32)
    tri = (np.arange(P)[:, None] <= np.arange(P)[None, :]).astype(f32)
    return {
        "xT": np.ascontiguousarray(np.asarray(x[b], f32).T),
        "wqT": np.ascontiguousarray((np.asarray(Wq, f32)[rows] * 0.125).T),
        "wkT": np.ascontiguousarray(np.asarray(Wk, f32)[rows].T),
        "wvT": np.ascontiguousarray(np.asarray(Wv, f32)[cols].T),
        "woT": np.ascontiguousarray(np.asarray(Wo, f32)[:, cols].T),
        "cosT": cosT,
        "sinT": sinT,
        "tri": tri,
    }


def get_nc():
    if "nc" not in _CACHE:
        _CACHE["nc"] = build_nc()
    return _CACHE["nc"]


def run_cores(in_maps, trace=False):
    from concourse.bass_utils import run_bass_kernel_spmd

    return run_bass_kernel_spmd(
        get_nc(), in_maps, core_ids=list(range(N_CORES)), trace=trace
    )


def combine(res):
    y = np.empty((B, S, D), np.float32)
    for b in range(B):
        y[b] = res.results[2 * b]["y"] + res.results[2 * b + 1]["y"]
    return y


def kernel(x, token_ids, Wq, Wk, Wv, Wo):
    in_maps = [
        prep_core_inputs(x, token_ids, Wq, Wk, Wv, Wo, c) for c in range(N_CORES)
    ]
    res = run_cores(in_maps)
    return combine(res)



# revision 9
# speedup vs baseline: 1.2835x; 1.2835x over previous
"""Causal multi-head self-attention (B=4, S=2048, D=1024, H=16) on 8 TRN2
NeuronCores.

Sharding: core c = (batch b = c//2, head-half = c%2). Each core computes, for
its batch and its 8 heads: QKV projections (+RoPE), causal softmax attention,
and a row-sharded output projection. The host sums the two partial y's per
batch.

v2 design (vs the f32r baseline):
  - fp16 everywhere on-chip except PSUM (fp32) and the y output. Data ranges
    are small (|score| < 5, den < 4e3), so fp16 is safe and gives 1 cyc/row
    matmuls at ANY moving width (no f32r <256 4x penalty), 2x DVE
    tensor_tensor, and half the DMA/SBUF footprint.
  - Single x pass: q, k, v projected from the same SBUF x slice.
  - RoPE: ScalarE copies the projection PSUM to fp16 SBUF (ACT is idle in
    this phase); the 6 DVE ops then all run at 2x fp16 rate.
  - Causal mask via PE: a [128,128] "identity @ (-60000*upper)" matmul
    accumulates -60000 into the masked region of the score PSUM before exp
    (exp -> 0), replacing per-block DVE mask multiplies.
  - Attention inner loop software-pipelined: scores for step i+1 issue
    before the AV matmul of step i, so the PE never waits on exp.
  - Denominator: v carries a 65th ones-column so AV emits the softmax
    denominator as PSUM row 64; reciprocal rows are broadcast across
    partitions with SBUF->SBUF DMA and applied as one fp16 multiply per
    (pair, query-block).
"""

import numpy as np

B, S, D = 4, 2048, 1024
NUM_HEADS = 16
THETA = 10000.0
DH = 64
N_CORES = 8
P = 128

_CACHE = {}


def build_nc():
    """Build the single-core SPMD Bass program (identical on all 8 cores)."""
    import concourse.mybir as mybir
    import concourse.tile as tile
    from concourse import bacc
    from concourse.bass import ts

    F16 = mybir.dt.float16
    F32 = mybir.dt.float32
    Act = mybir.ActivationFunctionType

    nc = bacc.Bacc(trn_type="TRN2")
    xT_d = nc.dram_tensor("xT", [D, S], F16, kind="ExternalInput")
    wqT_d = nc.dram_tensor("wqT", [D, 512], F16, kind="ExternalInput")
    wkT_d = nc.dram_tensor("wkT", [D, 512], F16, kind="ExternalInput")
    wvT_d = nc.dram_tensor("wvT", [D, 512], F16, kind="ExternalInput")
    woT_d = nc.dram_tensor("woT", [512, D], F16, kind="ExternalInput")
    cosT_d = nc.dram_tensor("cosT", [P, S], F16, kind="ExternalInput")
    sinT_d = nc.dram_tensor("sinT", [P, S], F16, kind="ExternalInput")
    mneg_d = nc.dram_tensor("mneg", [P, P], F16, kind="ExternalInput")
    iden_d = nc.dram_tensor("iden", [P, P], F16, kind="ExternalInput")
    y_d = nc.dram_tensor("y", [S, D], F32, kind="ExternalOutput")

    xT3 = xT_d.ap().rearrange("(kc p) s -> p kc s", p=P)     # [128, 8, 2048]
    wq3 = wqT_d.ap().rearrange("(kc p) j -> p kc j", p=P)    # [128, 8, 512]
    wk3 = wkT_d.ap().rearrange("(kc p) j -> p kc j", p=P)
    wv3 = wvT_d.ap().rearrange("(kc p) j -> p kc j", p=P)
    wo3 = woT_d.ap().rearrange("(jc p) i -> p jc i", p=P)    # [128, 4, 1024]
    y_ap = y_d.ap()

    with tile.TileContext(nc) as tc:
        with tc.tile_pool(name="pers", bufs=1) as pers:
            qT = pers.tile([P, 4, S], F16)
            kT = pers.tile([P, 4, S], F16)
            vA = pers.tile([P, 16, 8, 65], F16)
            outT = pers.tile([P, 4, S], F16)
            cosb = pers.tile([P, S], F16)
            sinb = pers.tile([P, S], F16)
            mneg = pers.tile([P, P], F16)
            iden = pers.tile([P, P], F16)

            nc.sync.dma_start(cosb[:], cosT_d.ap())
            nc.sync.dma_start(sinb[:], sinT_d.ap())
            nc.sync.dma_start(mneg[:], mneg_d.ap())
            nc.sync.dma_start(iden[:], iden_d.ap())

            # ---- Phase 1: q/k/v projections (+RoPE) in one x pass ----
            with (
                tc.tile_pool(name="w1", bufs=1) as w1,
                tc.tile_pool(name="x1", bufs=2) as x1,
                tc.tile_pool(name="sq", bufs=4) as sq,
                tc.tile_pool(name="tmp1", bufs=2) as tmp1,
                tc.tile_pool(name="ps1", bufs=4, space="PSUM") as ps1,
            ):
                wq_s = w1.tile([P, 8, 512], F16)
                wk_s = w1.tile([P, 8, 512], F16)
                wv_s = w1.tile([P, 8, 512], F16)
                for kc in range(8):  # chunked so kc=0 arrives early
                    nc.sync.dma_start(wq_s[:, kc, :], wq3[:, kc, :])
                    nc.sync.dma_start(wk_s[:, kc, :], wk3[:, kc, :])
                    nc.sync.dma_start(wv_s[:, kc, :], wv3[:, kc, :])
                nc.vector.memset(vA[:, :, :, 64:65], 1.0)

                def rope(pq, dst2d, sls):
                    # pq_s: fp16 copy of the projection PSUM (on ScalarE --
                    # idle in this phase); then 6 full-rate fp16 DVE ops.
                    pq_s = sq.tile([P, 512], F16, tag="pqs")
                    nc.scalar.copy(pq_s[:], pq[:])
                    tA = tmp1.tile([P, 512], F16, tag="tA")
                    nc.vector.tensor_mul(tA[:], pq_s[:], cosb[:, sls])
                    tBs = tmp1.tile([P, 512], F16, tag="tBs")
                    for hb in (0, 64):
                        nc.vector.tensor_mul(
                            tBs[hb : hb + 32, :],
                            pq_s[hb + 32 : hb + 64, :],
                            sinb[hb + 32 : hb + 64, sls],
                        )
                        nc.vector.tensor_mul(
                            tBs[hb + 32 : hb + 64, :],
                            pq_s[hb : hb + 32, :],
                            sinb[hb : hb + 32, sls],
                        )
                    nc.vector.tensor_add(dst2d, tA[:], tBs[:])

                for sl in range(4):
                    sls = ts(sl, 512)
                    xs = x1.tile([P, 8, 512], F16, tag="xs")
                    for kc in range(8):
                        nc.sync.dma_start(xs[:, kc, :], xT3[:, kc, sls])
                    for jc in range(4):
                        pq = ps1.tile([P, 512], F32, tag="p1")
                        for kc in range(8):
                            nc.tensor.matmul(
                                pq[:], wq_s[:, kc, ts(jc, P)], xs[:, kc, :],
                                start=(kc == 0), stop=(kc == 7),
                            )
                        rope(pq, qT[:, jc, sls], sls)
                        pk = ps1.tile([P, 512], F32, tag="p1")
                        for kc in range(8):
                            nc.tensor.matmul(
                                pk[:], wk_s[:, kc, ts(jc, P)], xs[:, kc, :],
                                start=(kc == 0), stop=(kc == 7),
                            )
                        rope(pk, kT[:, jc, sls], sls)
                    for t4 in range(4):
                        pv = ps1.tile([P, 512], F32, tag="p1")
                        for kc in range(8):
                            nc.tensor.matmul(
                                pv[:], xs[:, kc, ts(t4, P)], wv_s[:, kc, :],
                                start=(kc == 0), stop=(kc == 7),
                            )
                        nc.vector.tensor_copy(
                            vA[:, sl * 4 + t4, :, 0:64],
                            pv.rearrange("p (h c) -> p h c", h=8),
                        )

            # ---- Phase 2: attention, head-pair outer, SW-pipelined ----
            with (
                tc.tile_pool(name="ptp", bufs=3) as ptp,
                tc.tile_pool(name="rcp", bufs=4) as rcp,
                tc.tile_pool(name="rbp", bufs=3) as rbp,
                tc.tile_pool(name="wo", bufs=1) as wo,
                tc.tile_pool(name="ysb", bufs=2) as ysb,
                tc.tile_pool(name="drm", bufs=2, space="DRAM") as drm,
            ):
                _psB_cm = tc.tile_pool(name="psB", bufs=2, space="PSUM")
                _psC_cm = tc.tile_pool(name="psC", bufs=2, space="PSUM")
                psB = _psB_cm.__enter__()
                psC = _psC_cm.__enter__()
                wo_s = wo.tile([P, 4, D], F16)
                nc.sync.dma_start(wo_s[:], wo3)

                den_tiles = {}

                def sc_mms(hc, j, i):
                    # scores for both heads of the pair into one [P,1024]
                    # PSUM tile (head0 cols 0:512, head1 cols 512:1024),
                    # column-narrowed to the causal range; the diagonal
                    # 128-block gets -60000 added via the iden@mneg matmul.
                    m = i - 4 * j
                    w0 = max(m, 0) * P
                    diag = m >= 0
                    sc = psB.tile([P, 1024], F32, tag="sc")
                    for h01 in range(2):
                        hb, cs = h01 * 64, h01 * 512
                        nc.tensor.matmul(
                            sc[:, cs + w0 : cs + 512],
                            kT[hb : hb + 64, hc, ts(i, P)],
                            qT[hb : hb + 64, hc, j * 512 + w0 : (j + 1) * 512],
                            start=True, stop=not diag,
                        )
                        if diag:
                            nc.tensor.matmul(
                                sc[:, cs + w0 : cs + w0 + P],
                                iden[:], mneg[:],
                                start=False, stop=True,
                            )
                    return sc, w0

                def exp_pa(hc, j, i, sc, w0, pa, last):
                    pt = ptp.tile([P, 2, 512], F16, tag="pt")
                    sc2 = sc.rearrange("p (h q) -> p h q", h=2)
                    nc.scalar.activation(
                        pt[:, :, w0:512], sc2[:, :, w0:512], Act.Exp
                    )
                    for h01 in range(2):
                        nc.tensor.matmul(
                            pa[:, h01 * 512 + w0 : (h01 + 1) * 512],
                            vA[:, i, 2 * hc + h01, :],
                            pt[:, h01, w0:512],
                            start=(i == 0), stop=(i == last),
                        )

                def attention_pair(hc):
                    # DVE writes must start at a 32-aligned partition, so the
                    # 8 denominator rows stage through partition-0 srow tiles
                    # and a DRAM bounce before the batched reciprocal.
                    den_d = drm.tile([8, 512], F16, tag="dend")
                    den_tiles[hc] = den_d
                    for j in range(4):
                        pa = psC.tile([65, 1024], F32, tag="pa")
                        last = 4 * j + 3
                        sc_prev = sc_mms(hc, j, 0)
                        for i in range(last + 1):
                            sc_next = sc_mms(hc, j, i + 1) if i < last else None
                            exp_pa(hc, j, i, *sc_prev, pa, last)
                            sc_prev = sc_next
                        # epilogue: denominator rows + unnormalized out
                        for h01 in range(2):
                            cs = h01 * 512
                            r = 2 * j + h01
                            srow = rcp.tile([1, 512], F16, tag="srow")
                            nc.vector.tensor_copy(srow[:], pa[64:65, cs : cs + 512])
                            nc.sync.dma_start(den_d[r : r + 1, :], srow[:])
                            nc.vector.tensor_copy(
                                outT[h01 * 64 : h01 * 64 + 64, hc, ts(j, 512)],
                                pa[0:64, cs : cs + 512],
                            )

                def epilogue_pair(hc, p3=None):
                    # one 8-row reciprocal per pair; broadcast rows across
                    # partitions via SBUF->SBUF DMA; one fp16 multiply per j.
                    den_sb = rcp.tile([8, 512], F16, tag="densb")
                    nc.sync.dma_start(den_sb[:], den_tiles[hc][:])
                    rec8 = rcp.tile([8, 512], F16, tag="rec8")
                    with nc.allow_low_precision(reason="fp16 softmax denom"):
                        nc.vector.reciprocal(rec8[:], den_sb[:])
                    rec_d = drm.tile([8, 512], F16, tag="recd")
                    nc.sync.dma_start(rec_d[:], rec8[:])
                    for j in range(4):
                        rb = rbp.tile([P, 512], F16, tag="rb")
                        for h01 in range(2):
                            r = 2 * j + h01
                            nc.sync.dma_start(
                                rb[h01 * 64 : h01 * 64 + 64, :],
                                rec_d[r : r + 1, :].broadcast_to((64, 512)),
                            )
                        nc.vector.tensor_mul(
                            outT[:, hc, ts(j, 512)], outT[:, hc, ts(j, 512)], rb[:]
                        )
                        if p3 is not None:
                            p3(j)

                attention_pair(0)
                for hc in range(1, 4):
                    attention_pair(hc)
                    epilogue_pair(hc - 1)
                _psC_cm.__exit__(None, None, None)
                _psB_cm.__exit__(None, None, None)

                # ---- Phase 3: output projection y = outT.T @ woT ----
                ps3 = tc.tile_pool(name="ps3", bufs=2, space="PSUM")
                ps3p = ps3.__enter__()

                def p3_group(j):
                    for st in range(4 * j, 4 * j + 4):
                        py0 = ps3p.tile([P, 512], F32, tag="py0")
                        py1 = ps3p.tile([P, 512], F32, tag="py1")
                        for jc in range(4):
                            nc.tensor.matmul(
                                py0[:], outT[:, jc, ts(st, P)],
                                wo_s[:, jc, 0:512],
                                start=(jc == 0), stop=(jc == 3),
                            )
                        for jc in range(4):
                            nc.tensor.matmul(
                                py1[:], outT[:, jc, ts(st, P)],
                                wo_s[:, jc, 512:D],
                                start=(jc == 0), stop=(jc == 3),
                            )
                        yo0 = ysb.tile([P, 512], F32, tag="yo0")
                        yo1 = ysb.tile([P, 512], F32, tag="yo1")
                        nc.scalar.copy(yo0[:], py0[:])
                        nc.scalar.copy(yo1[:], py1[:])
                        nc.sync.dma_start(y_ap[ts(st, P), 0:512], yo0[:])
                        nc.sync.dma_start(y_ap[ts(st, P), 512:D], yo1[:])

                epilogue_pair(3)
                for _j in range(4):
                    p3_group(_j)
                ps3.__exit__(None, None, None)

    nc.compile()
    return nc


def prep_core_inputs(x, token_ids, Wq, Wk, Wv, Wo, core):
    b, half = divmod(core, 2)
    rows = []
    for h in range(half * 8, half * 8 + 8):
        base = h * DH
        rows.extend(base + np.arange(0, DH, 2))
        rows.extend(base + np.arange(1, DH, 2))
    rows = np.asarray(rows)
    cols = np.arange(half * 512, half * 512 + 512)

    f16 = np.float16
    f32 = np.float32
    inv = THETA ** (-np.arange(0, DH, 2, dtype=np.float64) / DH)
    ang = np.asarray(token_ids, dtype=np.float64)[None, :] * inv[:, None]
    cosT = np.tile(np.cos(ang), (4, 1)).astype(f16)
    sin_block = np.concatenate([np.sin(ang), -np.sin(ang)], axis=0)
    sinT = np.tile(sin_block, (2, 1)).astype(f16)
    mneg = np.where(
        np.arange(P)[:, None] > np.arange(P)[None, :], f32(-60000.0), f32(0.0)
    ).astype(f16)
    iden = np.eye(P, dtype=f16)
    return {
        "xT": np.ascontiguousarray(np.asarray(x[b], f32).T.astype(f16)),
        "wqT": np.ascontiguousarray((np.asarray(Wq, f32)[rows] * 0.125).T.astype(f16)),
        "wkT": np.ascontiguousarray(np.asarray(Wk, f32)[rows].T.astype(f16)),
        "wvT": np.ascontiguousarray(np.asarray(Wv, f32)[cols].T.astype(f16)),
        "woT": np.ascontiguousarray(np.asarray(Wo, f32)[:, cols].T.astype(f16)),
        "cosT": cosT,
        "sinT": sinT,
        "mneg": mneg,
        "iden": iden,
    }


def get_nc():
    if "nc" not in _CACHE:
        _CACHE["nc"] = build_nc()
    return _CACHE["nc"]


def run_cores(in_maps, trace=False):
    from concourse.bass_utils import run_bass_kernel_spmd

    return run_bass_kernel_spmd(
        get_nc(), in_maps, core_ids=list(range(N_CORES)), trace=trace
    )


def combine(res):
    y = np.empty((B, S, D), np.float32)
    for b in range(B):
        y[b] = res.results[2 * b]["y"] + res.results[2 * b + 1]["y"]
    return y


def kernel(x, token_ids, Wq, Wk, Wv, Wo):
    in_maps = [
        prep_core_inputs(x, token_ids, Wq, Wk, Wv, Wo, c) for c in range(N_CORES)
    ]
    res = run_cores(in_maps)
    return combine(res)
